# revision 10
# baseline (speedup 1.0000x reference)
"""Bahdanau additive attention on 8 Trainium2 NeuronCores.

Problem: B=32, S=1024, H=1024 fp32.
  U_h   = dec @ U_w.T                    [B, H]
  W_s   = enc @ W_w.T                    [B, S, H]
  att   = tanh(U_h[:,None,:] + W_s) @ v  [B, S]
  alpha = softmax(att, axis=1)
  ctx   = einsum('bs,bsh->bh', alpha, enc)

Sharding: data-parallel over B across 8 cores (4 batches per core),
U_w / W_w / v_w replicated.

Per-core kernel (all matmuls in bf16 with fp32 PSUM accumulation):
  - enc arrives through a handful of large SWDGE cast-DMAs (f32 DRAM ->
    bf16 SBUF directly, no staging or DVE casts) on the gpsimd queue:
    batch 0 in two halves, batch 1 in two halves mid-batch-0, batches
    2/3 whole at the starts of batches 1/2.
  - W_w / U_w stream on the sync HWDGE queue as f32 row-pair loads in
    just-in-time order (W pair p before U pair p), DVE-cast, then
    PE-transposed; U_h o-tiles ride the same arrivals.
  - dummy matmuls on a zero tile warm the PE HAM clock gate early.
  - the W_s loop is s-chunk-major (c = s-half outer, o-tile i inner):
    batch 0's c=0 phase starts once enc[0]'s first half + W pair 0 have
    landed (~10us) instead of after the whole 12MB prologue.
  - next-batch enc PE-transposes run only in the c=1 phase (2 groups
    per o-tile) so they never wait on in-flight DMA during c=0.
  - ScalarE tanh with per-partition bias U_hT[o] on PSUM evacuation;
    v-matvec and ctx matmuls use 4-way column-group concurrency
    (N=256 quarters at tile_position (0,32q)).
  - per-batch softmax on a partition-0 staging row; alpha bounced
    through DRAM (sync queue) to transpose it for the context matmul
    (PE transpose for the last batch).
"""

import numpy as np
from contextlib import ExitStack

import concourse.bacc as bacc
import concourse.mybir as mybir
import concourse.tile as tile
from concourse import masks
from concourse.bass_utils import run_bass_kernel_spmd

N_CORES = 8
B = 32
B_L = B // N_CORES  # 4 batches per core
S = 1024
H = 1024
P = 128
NT = 8  # 1024 / 128 tiles
F32 = mybir.dt.float32
BF16 = mybir.dt.bfloat16
AF = mybir.ActivationFunctionType

WARM0 = 24  # initial HAM warm-up matmuls (N=256)
WARM_K = 2  # maintenance warm matmuls per b0 c0 slot


def _emit(tc, stop_after="full"):
    nc = tc.nc
    dec = nc.dram_tensor("dec", [B_L, H], F32, kind="ExternalInput").ap()
    enc = nc.dram_tensor("enc", [B_L, S, H], F32, kind="ExternalInput").ap()
    U_w = nc.dram_tensor("U_w", [H, H], F32, kind="ExternalInput").ap()
    W_w = nc.dram_tensor("W_w", [H, H], F32, kind="ExternalInput").ap()
    v_w = nc.dram_tensor("v_w", [H], F32, kind="ExternalInput").ap()
    ctx_out = nc.dram_tensor("ctx", [B_L, H], F32, kind="ExternalOutput").ap()
    alpha_out = nc.dram_tensor("alpha", [B_L, S], F32, kind="ExternalOutput").ap()

    ctx = ExitStack()
    const = ctx.enter_context(tc.tile_pool(name="const", bufs=1))
    natp = ctx.enter_context(tc.tile_pool(name="nat", bufs=4))
    encTp = ctx.enter_context(tc.tile_pool(name="encT", bufs=2))
    tanhp = ctx.enter_context(tc.tile_pool(name="tanh", bufs=3))
    stgp = ctx.enter_context(tc.tile_pool(name="stg", bufs=1))
    mmps = ctx.enter_context(tc.tile_pool(name="mmps", bufs=4, space="PSUM"))
    trps = ctx.enter_context(tc.tile_pool(name="trps", bufs=2, space="PSUM"))
    attps = ctx.enter_context(tc.tile_pool(name="attps", bufs=1, space="PSUM"))

    identb = const.tile([P, P], BF16)
    masks.make_identity(nc, identb[:])
    identf = const.tile([P, P], F32)  # built later (only b3 epilogue needs it)

    # --- HAM warm-up: dummy matmuls on a zeroed tile keep the PE clock
    # gate at 2.4 GHz while the prologue DMAs stream in.
    scr = const.tile([P, 256], BF16)
    nc.gpsimd.memset(scr[:], 0.0)

    def warm(n):
        for _ in range(n):
            wps = trps.tile([P, 256], F32, tag="tr", name="warm_ps")
            nc.tensor.matmul(
                wps[:], scr[:, 0:P], scr[:], start=True, stop=True
            )

    warm(WARM0)

    # --- sync queue: dec + v first (tiny, needed early) ---
    dec_nat = const.tile([B_L, H], F32)
    nc.sync.dma_start(dec_nat[:], dec[:])
    vT_f = const.tile([P, NT], F32)
    nc.sync.dma_start(vT_f[:], v_w.rearrange("(t p) -> p t", p=P))

    # --- persistent SBUF tensors ---
    W_wT = const.tile([P, NT, H], BF16)  # [h_in, h_tile, o]
    U_wT = encTp.tile([P, NT, H], BF16, tag="encT")
    U_hT = const.tile([P, NT, B_L], F32)
    encN = const.tile([P, B_L, NT, H], BF16)  # [s_in, b, s_tile, h]
    alphaT = const.tile([P, NT, B_L], BF16)
    encT_cur = encTp.tile([P, NT, S], BF16, tag="encT", name="encT_0")

    # --- gpsimd queue: enc batch 0 via two cast-DMA halves ---
    def emit_enc_load(b, k0, nk):
        """SWDGE cast-DMA: enc[b] s-chunks [k0, k0+nk) f32 -> encN bf16."""
        nc.gpsimd.dma_start(
            encN[:, b, k0 : k0 + nk, :],
            enc[b, k0 * P : (k0 + nk) * P, :].rearrange(
                "(k p) h -> p k h", p=P
            ),
        )

    emit_enc_load(0, 0, 4)
    emit_enc_load(0, 4, 4)

    dec_b16 = const.tile([B_L, H], BF16)
    nc.vector.tensor_copy(dec_b16[:], dec_nat[:])
    decT = const.tile([P, NT, B_L], BF16)
    for k in range(NT):
        ps = mmps.tile([P, B_L], BF16, tag="mm", name="dec_ps")
        nc.tensor.transpose(
            ps[:], dec_b16[:, k * P : (k + 1) * P], identb[0:B_L, 0:B_L]
        )
        nc.vector.tensor_copy(decT[:, k, :], ps[:])
    vT = const.tile([P, NT], BF16)
    nc.vector.tensor_copy(vT[:], vT_f[:])

    def emit_weight_pair(dst, src_dram, p):
        """Sync-queue f32 load of rows [2p,2p+2) of a [1024,1024] weight,
        DVE cast, PE-transpose into dst[:, :, 2p*128:(2p+2)*128]."""
        natt = natp.tile([P, 2, H], F32, tag="nat", name="natw")
        nc.sync.dma_start(
            natt[:],
            src_dram[2 * p * P : (2 * p + 2) * P, :].rearrange(
                "(r p) h -> p r h", p=P
            ),
        )
        natb = natp.tile([P, 2, H], BF16, tag="natb")
        nc.vector.tensor_copy(natb[:], natt[:])
        for r in range(2):
            i = 2 * p + r
            for g in range(2):
                ps = trps.tile([P, 4, P], BF16, tag="tr", name="wtr_ps")
                for jj in range(4):
                    j = 4 * g + jj
                    nc.tensor.transpose(
                        ps[:, jj, :], natb[:, r, j * P : (j + 1) * P], identb[:]
                    )
                nc.vector.tensor_copy(
                    dst[:, 4 * g : 4 * g + 4, i * P : (i + 1) * P], ps[:]
                )

    def emit_transpose_group(encT_b, b, k, g):
        """PE-transpose 4 h-tiles of chunk (b, k) into encT_b."""
        ps = trps.tile([P, 4, P], BF16, tag="tr")
        for jj in range(4):
            j = 4 * g + jj
            nc.tensor.transpose(
                ps[:, jj, :], encN[:, b, k, j * P : (j + 1) * P], identb[:]
            )
        nc.vector.tensor_copy(
            encT_b[:, 4 * g : 4 * g + 4, k * P : (k + 1) * P], ps[:]
        )

    def emit_uh(i):
        """U_hT[:, i, :] = sum_j U_wT[:, j, i-slice].T @ decT[:, j, :]."""
        ps = mmps.tile([P, B_L], F32, tag="mm", name="uh_ps")
        for j in range(NT):
            nc.tensor.matmul(
                ps[:],
                U_wT[:, j, i * P : (i + 1) * P],
                decT[:, j, :],
                start=(j == 0),
                stop=(j == NT - 1),
            )
        nc.vector.tensor_copy(U_hT[:, i, :], ps[:])

    def emit_ctx(b):
        """Context matmuls (4-way col groups) + evacuation + output DMA."""
        ps = attps.tile([P, 256], F32, tag="att1", name="ctx_ps")
        for k in range(NT):
            lhsT = alphaT[:, k, b : b + 1]
            for q in range(4):
                nc.tensor.matmul(
                    ps[32 * q : 32 * q + 1, :],
                    lhsT,
                    encN[:, b, k, 256 * q : 256 * (q + 1)],
                    start=(k == 0),
                    stop=(k == NT - 1),
                    tile_position=(0, 32 * q),
                )
        ctx_stg = stgp.tile([1, H], F32, tag="ctxstg")
        for q in range(4):
            nc.vector.tensor_copy(
                ctx_stg[0:1, 256 * q : 256 * (q + 1)],
                ps[32 * q : 32 * q + 1, :],
            )
        nc.sync.dma_start(ctx_out[b : b + 1, :], ctx_stg[0:1, :])

    if stop_after == "setup":
        dbg = stgp.tile([1, S], F32, tag="ctxstg")
        nc.gpsimd.memset(dbg[:], 0.0)
        for b in range(B_L):
            nc.sync.dma_start(alpha_out[b : b + 1, :], dbg[0:1, :])
            nc.sync.dma_start(ctx_out[b : b + 1, :], dbg[0:1, 0:H])
        ctx.close()
        return

    def emit_matvec(att_ps, c, i, th):
        for h in range(2):
            nc.tensor.matmul(
                att_ps[64 * c + 32 * h : 64 * c + 32 * h + 1, :],
                vT[:, i : i + 1],
                th[:, 256 * h : 256 * (h + 1)],
                start=(i == 0),
                stop=(i == NT - 1),
                tile_position=(0, 64 * c + 32 * h),
            )

    def emit_softmax_epilogue(b):
        att_ps = att_tiles[b]
        att_stg = stgp.tile([1, S], F32, tag="attstg")
        for q in range(4):
            nc.vector.tensor_copy(
                att_stg[0:1, 256 * q : 256 * (q + 1)],
                att_ps[32 * q : 32 * q + 1, :],
            )
        if stop_after != "phase1":
            smax = stgp.tile([1, 1], F32, tag="smax")
            nc.vector.reduce_max(smax[:], att_stg[:], axis=mybir.AxisListType.X)
            negmax = stgp.tile([1, 1], F32, tag="negmax")
            nc.vector.tensor_scalar_mul(negmax[:], smax[:], -1.0)
            exp_stg = stgp.tile([1, S], F32, tag="expstg")
            ssum = stgp.tile([1, 1], F32, tag="ssum")
            nc.scalar.activation(
                exp_stg[:],
                att_stg[:],
                AF.Exp,
                bias=negmax[:],
                scale=1.0,
                accum_out=ssum[:],
            )
            srec = stgp.tile([1, 1], F32, tag="srec")
            nc.vector.reciprocal(srec[:], ssum[:])
            alpha_stg = stgp.tile([1, S], F32, tag="alphastg")
            nc.vector.tensor_scalar_mul(alpha_stg[:], exp_stg[:], srec[:])
        else:
            alpha_stg = att_stg
        nc.sync.dma_start(alpha_out[b : b + 1, :], alpha_stg[0:1, :])
        if b < B_L - 1:
            # bounce through DRAM to transpose alpha (latency hidden here)
            alphaT_f = stgp.tile([P, NT], F32, tag="alphaTf")
            nc.sync.dma_start(
                alphaT_f[:], alpha_out[b].rearrange("(k p) -> p k", p=P)
            )
            nc.vector.tensor_copy(alphaT[:, :, b], alphaT_f[:])
        else:
            # last batch: PE is idle; transpose alpha on the array instead
            ps = mmps.tile([P, NT], F32, tag="mm", name="alpha_ps")
            for k in range(NT):
                nc.tensor.transpose(
                    ps[:, k : k + 1],
                    alpha_stg[0:1, k * P : (k + 1) * P],
                    identf[0:1, 0:1],
                )
            nc.vector.tensor_copy(alphaT[:, :, b], ps[:])

    att_tiles = {}

    # --- enc[0] chunks 0-3 transposes: needed before b0's c=0 phase ---
    for k in range(4):
        for g in range(2):
            emit_transpose_group(encT_cur, 0, k, g)
        warm(WARM_K)

    # ================= batch loop, s-chunk-major =================
    for b in range(B_L):
        encT_next = None
        if b + 1 < B_L:
            encT_next = encTp.tile(
                [P, NT, S], BF16, tag="encT", name=f"encT_{b + 1}"
            )
        att_ps = attps.tile([P, 256], F32, tag="att0", name="att_ps")
        att_tiles[b] = att_ps

        for c in range(2):
            tanh_prev = None
            for i in range(NT):
                if b == 0 and c == 0:
                    warm(WARM_K)
                    if i % 2 == 0:
                        # fused weight stream: W pair gates these two
                        # o-tiles' matmuls, U pair + U_h ride behind.
                        emit_weight_pair(W_wT, W_w, i // 2)
                ps = mmps.tile([P, 512], F32, tag="mm", name="mm_ps")
                for j in range(NT):
                    nc.tensor.matmul(
                        ps[:],
                        W_wT[:, j, i * P : (i + 1) * P],
                        encT_cur[:, j, c * 512 : (c + 1) * 512],
                        start=(j == 0),
                        stop=(j == NT - 1),
                    )
                if b == 0 and c == 0:
                    if i % 2 == 0:
                        emit_weight_pair(U_wT, U_w, i // 2)
                        emit_uh(i)
                        emit_uh(i + 1)
                    if i == 4:
                        emit_enc_load(1, 0, 4)
                    if i == 6:
                        emit_enc_load(1, 4, 4)
                    if i >= 5:
                        # enc[0] chunks 4-7 transposes for the c=1 phase
                        start = 3 * (i - 5)
                        cnt = 3 if i < 7 else 2
                        for e in range(start, start + cnt):
                            k, g = divmod(8 + e, 2)
                            emit_transpose_group(encT_cur, 0, k, g)
                else:
                    if c == 0 and b == 1 and i == 0:
                        emit_enc_load(2, 0, 8)
                    if c == 0 and b == 2 and i == 0:
                        emit_enc_load(3, 0, 8)
                    if c == 0 and i == 3 and b > 0:
                        emit_ctx(b - 1)
                    if c == 1 and encT_next is not None:
                        for e in range(2):
                            k, g = divmod(2 * i + e, 2)
                            emit_transpose_group(encT_next, b + 1, k, g)
                if tanh_prev is not None:
                    ip, thp = tanh_prev
                    emit_matvec(att_ps, c, ip, thp)
                th = tanhp.tile([P, 512], BF16, tag="tanh")
                nc.scalar.activation(
                    th[:],
                    ps[:],
                    AF.Tanh,
                    bias=U_hT[:, i, b : b + 1],
                    scale=1.0,
                )
                tanh_prev = (i, th)
            ip, thp = tanh_prev
            emit_matvec(att_ps, c, ip, thp)

        if b == 0:
            masks.make_identity(nc, identf[:])
        emit_softmax_epilogue(b)
        if encT_next is not None:
            encT_cur = encT_next

    emit_ctx(B_L - 1)
    ctx.close()


_CACHED = None


def _build(stop_after="full"):
    global _CACHED
    if _CACHED is None:
        nc = bacc.Bacc("TRN2", target_bir_lowering=False, debug=False)
        with tile.TileContext(nc) as tc:
            _emit(tc, stop_after=stop_after)
        nc.compile()
        _CACHED = nc
    return _CACHED


def kernel(
    decoder_hidden: np.ndarray,
    encoder_outputs: np.ndarray,
    U_w: np.ndarray,
    W_w: np.ndarray,
    v_w: np.ndarray,
):
    dec = np.ascontiguousarray(np.asarray(decoder_hidden, dtype=np.float32))
    enc = np.ascontiguousarray(np.asarray(encoder_outputs, dtype=np.float32))
    U = np.ascontiguousarray(np.asarray(U_w, dtype=np.float32))
    W = np.ascontiguousarray(np.asarray(W_w, dtype=np.float32))
    v = np.ascontiguousarray(np.asarray(v_w, dtype=np.float32))

    nc = _build()
    in_maps = []
    for c in range(N_CORES):
        sl = slice(c * B_L, (c + 1) * B_L)
        in_maps.append(
            {"dec": dec[sl], "enc": enc[sl], "U_w": U, "W_w": W, "v_w": v}
        )
    res = run_bass_kernel_spmd(nc, in_maps, core_ids=list(range(N_CORES)))
    context = np.concatenate([res.results[c]["ctx"] for c in range(N_CORES)], axis=0)
    alpha = np.concatenate([res.results[c]["alpha"] for c in range(N_CORES)], axis=0)
    return (context.astype(np.float32), alpha.astype(np.float32))


# revision 11
# speedup vs baseline: 1.2706x; 1.2706x over previous
"""Bahdanau additive attention on 8 Trainium2 NeuronCores.

Problem: B=32, S=1024, H=1024 fp32.
  U_h   = dec @ U_w.T                    [B, H]
  W_s   = enc @ W_w.T                    [B, S, H]
  att   = tanh(U_h[:,None,:] + W_s) @ v  [B, S]
  alpha = softmax(att, axis=1)
  ctx   = einsum('bs,bsh->bh', alpha, enc)

Sharding: data-parallel over B across 8 cores (4 batches per core),
U_w / W_w / v_w replicated.

Per-core kernel (all matmuls in bf16 with fp32 PSUM accumulation):
  - ONE sync-HWDGE DMA queue carries every f32 input in just-in-time
    order: dec, v, enc[0] quarter-loads (1MB each), then W/U row-pair
    loads interleaved with enc[1] quarters, then enc[b+1] during batch
    b and the alpha/ctx output traffic.  Large batched transfers keep
    the stream bandwidth-bound instead of instruction-issue-bound.
  - dummy matmuls on a zero tile warm the PE HAM clock gate (1.2 ->
    2.4 GHz) from ~4us so real matmuls never run at half clock.
  - batch 0's o-tile loop is fused with the weight stream: o-tiles
    2p/2p+1 run as soon as W row-pair p lands; U pairs + U_h ride
    behind on the same arrivals.
  - batches 1-3: per-slot interleave of enc[b+1] quarter loads, DVE
    casts, PE transposes (2 groups per slot), and ctx(b-1).
  - ScalarE tanh with per-partition bias U_hT[o] on PSUM evacuation;
    v-matvec and ctx matmuls use 4-way column-group concurrency
    (N=256 quarters at tile_position (0,32q)).
  - per-batch softmax on a partition-0 staging row; alpha bounced
    through DRAM to transpose it for the context matmul (PE transpose
    for the last batch).
"""

import numpy as np
from contextlib import ExitStack

import concourse.bacc as bacc
import concourse.mybir as mybir
import concourse.tile as tile
from concourse import masks
from concourse.bass_utils import run_bass_kernel_spmd

N_CORES = 8
B = 32
B_L = B // N_CORES  # 4 batches per core
S = 1024
H = 1024
P = 128
NT = 8  # 1024 / 128 tiles
F32 = mybir.dt.float32
BF16 = mybir.dt.bfloat16
AF = mybir.ActivationFunctionType

WARM0 = 24  # initial HAM warm-up matmuls (N=256)
WARM_K = 2  # maintenance warm matmuls per prologue step


def _emit(tc, stop_after="full"):
    nc = tc.nc
    dec = nc.dram_tensor("dec", [B_L, H], F32, kind="ExternalInput").ap()
    enc = nc.dram_tensor("enc", [B_L, S, H], F32, kind="ExternalInput").ap()
    U_w = nc.dram_tensor("U_w", [H, H], F32, kind="ExternalInput").ap()
    W_w = nc.dram_tensor("W_w", [H, H], F32, kind="ExternalInput").ap()
    v_w = nc.dram_tensor("v_w", [H], F32, kind="ExternalInput").ap()
    ctx_out = nc.dram_tensor("ctx", [B_L, H], F32, kind="ExternalOutput").ap()
    alpha_out = nc.dram_tensor("alpha", [B_L, S], F32, kind="ExternalOutput").ap()

    ctx = ExitStack()
    const = ctx.enter_context(tc.tile_pool(name="const", bufs=1))
    natp = ctx.enter_context(tc.tile_pool(name="nat", bufs=2))
    encq = ctx.enter_context(tc.tile_pool(name="encq", bufs=3))
    encTp = ctx.enter_context(tc.tile_pool(name="encT", bufs=2))
    tanhp = ctx.enter_context(tc.tile_pool(name="tanh", bufs=3))
    stgp = ctx.enter_context(tc.tile_pool(name="stg", bufs=1))
    mmps = ctx.enter_context(tc.tile_pool(name="mmps", bufs=4, space="PSUM"))
    trps = ctx.enter_context(tc.tile_pool(name="trps", bufs=2, space="PSUM"))
    attps = ctx.enter_context(tc.tile_pool(name="attps", bufs=1, space="PSUM"))

    identb = const.tile([P, P], BF16)
    masks.make_identity(nc, identb[:])
    identf = const.tile([P, P], F32)  # built later (only b3 epilogue needs it)

    # --- HAM warm-up: dummy matmuls on a zeroed tile keep the PE clock
    # gate at 2.4 GHz while the prologue DMAs stream in.
    scr = const.tile([P, 256], BF16)
    nc.gpsimd.memset(scr[:], 0.0)

    def warm(n):
        for _ in range(n):
            wps = trps.tile([P, 256], F32, tag="tr", name="warm_ps")
            nc.tensor.matmul(
                wps[:], scr[:, 0:P], scr[:], start=True, stop=True
            )

    warm(WARM0)

    # --- sync queue: dec + v first (tiny, needed early) ---
    dec_nat = const.tile([B_L, H], F32)
    nc.sync.dma_start(dec_nat[:], dec[:])
    vT_f = const.tile([P, NT], F32)
    nc.sync.dma_start(vT_f[:], v_w.rearrange("(t p) -> p t", p=P))

    # --- persistent SBUF tensors ---
    W_wT = const.tile([P, NT, H], BF16)  # [h_in, h_tile, o]
    U_wT = encTp.tile([P, NT, H], BF16, tag="encT")
    U_hT = const.tile([P, NT, B_L], F32)
    encN = const.tile([P, B_L, NT, H], BF16)  # [s_in, b, s_tile, h]
    alphaT = const.tile([P, NT, B_L], BF16)
    encT_cur = encTp.tile([P, NT, S], BF16, tag="encT", name="encT_0")

    def emit_enc_quarter(b, kk):
        """1MB f32 load of enc[b] s-chunks [2kk, 2kk+2) + DVE cast."""
        natt = encq.tile([P, 2, H], F32, tag="encq", name="enc_stg")
        nc.sync.dma_start(
            natt[:],
            enc[b, 2 * kk * P : (2 * kk + 2) * P, :].rearrange(
                "(k p) h -> p k h", p=P
            ),
        )
        nc.vector.tensor_copy(encN[:, b, 2 * kk : 2 * kk + 2, :], natt[:])

    emit_enc_quarter(0, 0)
    emit_enc_quarter(0, 1)
    emit_enc_quarter(0, 2)
    emit_enc_quarter(0, 3)

    dec_b16 = const.tile([B_L, H], BF16)
    nc.vector.tensor_copy(dec_b16[:], dec_nat[:])
    decT = const.tile([P, NT, B_L], BF16)
    for k in range(NT):
        ps = mmps.tile([P, B_L], BF16, tag="mm", name="dec_ps")
        nc.tensor.transpose(
            ps[:], dec_b16[:, k * P : (k + 1) * P], identb[0:B_L, 0:B_L]
        )
        nc.vector.tensor_copy(decT[:, k, :], ps[:])
    vT = const.tile([P, NT], BF16)
    nc.vector.tensor_copy(vT[:], vT_f[:])

    def emit_weight_pair(dst, src_dram, p):
        """Sync-queue f32 load of rows [2p,2p+2) of a [1024,1024] weight,
        DVE cast, PE-transpose into dst[:, :, 2p*128:(2p+2)*128]."""
        natt = natp.tile([P, 2, H], F32, tag="nat", name="natw")
        nc.sync.dma_start(
            natt[:],
            src_dram[2 * p * P : (2 * p + 2) * P, :].rearrange(
                "(r p) h -> p r h", p=P
            ),
        )
        natb = natp.tile([P, 2, H], BF16, tag="natb")
        nc.vector.tensor_copy(natb[:], natt[:])
        for r in range(2):
            i = 2 * p + r
            for g in range(2):
                ps = trps.tile([P, 4, P], BF16, tag="tr", name="wtr_ps")
                for jj in range(4):
                    j = 4 * g + jj
                    nc.tensor.transpose(
                        ps[:, jj, :], natb[:, r, j * P : (j + 1) * P], identb[:]
                    )
                nc.vector.tensor_copy(
                    dst[:, 4 * g : 4 * g + 4, i * P : (i + 1) * P], ps[:]
                )

    def emit_transpose_group(encT_b, b, k, g):
        """PE-transpose 4 h-tiles of chunk (b, k) into encT_b."""
        ps = trps.tile([P, 4, P], BF16, tag="tr")
        for jj in range(4):
            j = 4 * g + jj
            nc.tensor.transpose(
                ps[:, jj, :], encN[:, b, k, j * P : (j + 1) * P], identb[:]
            )
        nc.vector.tensor_copy(
            encT_b[:, 4 * g : 4 * g + 4, k * P : (k + 1) * P], ps[:]
        )

    def emit_uh(i):
        """U_hT[:, i, :] = sum_j U_wT[:, j, i-slice].T @ decT[:, j, :]."""
        ps = mmps.tile([P, B_L], F32, tag="mm", name="uh_ps")
        for j in range(NT):
            nc.tensor.matmul(
                ps[:],
                U_wT[:, j, i * P : (i + 1) * P],
                decT[:, j, :],
                start=(j == 0),
                stop=(j == NT - 1),
            )
        nc.vector.tensor_copy(U_hT[:, i, :], ps[:])

    def emit_ctx(b):
        """Context matmuls (4-way col groups) + evacuation + output DMA."""
        ps = attps.tile([P, 256], F32, tag="att1", name="ctx_ps")
        for k in range(NT):
            lhsT = alphaT[:, k, b : b + 1]
            for q in range(4):
                nc.tensor.matmul(
                    ps[32 * q : 32 * q + 1, :],
                    lhsT,
                    encN[:, b, k, 256 * q : 256 * (q + 1)],
                    start=(k == 0),
                    stop=(k == NT - 1),
                    tile_position=(0, 32 * q),
                )
        ctx_stg = stgp.tile([1, H], F32, tag="ctxstg")
        for q in range(4):
            nc.vector.tensor_copy(
                ctx_stg[0:1, 256 * q : 256 * (q + 1)],
                ps[32 * q : 32 * q + 1, :],
            )
        nc.sync.dma_start(ctx_out[b : b + 1, :], ctx_stg[0:1, :])

    if stop_after == "setup":
        dbg = stgp.tile([1, S], F32, tag="ctxstg")
        nc.gpsimd.memset(dbg[:], 0.0)
        for b in range(B_L):
            nc.sync.dma_start(alpha_out[b : b + 1, :], dbg[0:1, :])
            nc.sync.dma_start(ctx_out[b : b + 1, :], dbg[0:1, 0:H])
        ctx.close()
        return

    def emit_matvec(att_ps, i, th):
        for q in range(4):
            nc.tensor.matmul(
                att_ps[32 * q : 32 * q + 1, :],
                vT[:, i : i + 1],
                th[:, 256 * q : 256 * (q + 1)],
                start=(i == 0),
                stop=(i == NT - 1),
                tile_position=(0, 32 * q),
            )

    def emit_softmax_epilogue(b):
        att_ps = att_tiles[b]
        att_stg = stgp.tile([1, S], F32, tag="attstg")
        for q in range(4):
            nc.vector.tensor_copy(
                att_stg[0:1, 256 * q : 256 * (q + 1)],
                att_ps[32 * q : 32 * q + 1, :],
            )
        if stop_after != "phase1":
            smax = stgp.tile([1, 1], F32, tag="smax")
            nc.vector.reduce_max(smax[:], att_stg[:], axis=mybir.AxisListType.X)
            negmax = stgp.tile([1, 1], F32, tag="negmax")
            nc.vector.tensor_scalar_mul(negmax[:], smax[:], -1.0)
            exp_stg = stgp.tile([1, S], F32, tag="expstg")
            ssum = stgp.tile([1, 1], F32, tag="ssum")
            nc.scalar.activation(
                exp_stg[:],
                att_stg[:],
                AF.Exp,
                bias=negmax[:],
                scale=1.0,
                accum_out=ssum[:],
            )
            srec = stgp.tile([1, 1], F32, tag="srec")
            nc.vector.reciprocal(srec[:], ssum[:])
            alpha_stg = stgp.tile([1, S], F32, tag="alphastg")
            nc.vector.tensor_scalar_mul(alpha_stg[:], exp_stg[:], srec[:])
        else:
            alpha_stg = att_stg
        nc.sync.dma_start(alpha_out[b : b + 1, :], alpha_stg[0:1, :])
        if b < B_L - 1:
            # bounce through DRAM to transpose alpha (latency hidden here)
            alphaT_f = stgp.tile([P, NT], F32, tag="alphaTf")
            nc.sync.dma_start(
                alphaT_f[:], alpha_out[b].rearrange("(k p) -> p k", p=P)
            )
            nc.vector.tensor_copy(alphaT[:, :, b], alphaT_f[:])
        else:
            # last batch: PE is idle; transpose alpha on the array instead
            ps = mmps.tile([P, NT], F32, tag="mm", name="alpha_ps")
            for k in range(NT):
                nc.tensor.transpose(
                    ps[:, k : k + 1],
                    alpha_stg[0:1, k * P : (k + 1) * P],
                    identf[0:1, 0:1],
                )
            nc.vector.tensor_copy(alphaT[:, :, b], ps[:])

    att_tiles = {}

    # --- enc[0] transposes: all 8 chunks, before b0's loop ---
    for k in range(NT):
        for g in range(2):
            emit_transpose_group(encT_cur, 0, k, g)
        warm(WARM_K)

    # ================= batch loop, o-tile-major =================
    for b in range(B_L):
        encT_next = None
        if b + 1 < B_L:
            encT_next = encTp.tile(
                [P, NT, S], BF16, tag="encT", name=f"encT_{b + 1}"
            )
        att_ps = attps.tile([P, 256], F32, tag="att0", name="att_ps")
        att_tiles[b] = att_ps

        tanh_prev = None
        for i in range(NT):
            if b == 0 and i % 2 == 0:
                # fused weight stream: W pair gates o-tiles 2p/2p+1
                emit_weight_pair(W_wT, W_w, i // 2)
            ps = [
                mmps.tile([P, 512], F32, tag="mm", name=f"mm_ps{c2}")
                for c2 in range(2)
            ]
            for j in range(NT):
                lhsT = W_wT[:, j, i * P : (i + 1) * P]
                for c in range(2):
                    nc.tensor.matmul(
                        ps[c][:],
                        lhsT,
                        encT_cur[:, j, c * 512 : (c + 1) * 512],
                        start=(j == 0),
                        stop=(j == NT - 1),
                    )
            if b == 0:
                if i % 2 == 0:
                    emit_weight_pair(U_wT, U_w, i // 2)
                    emit_uh(i)
                    emit_uh(i + 1)
                else:
                    emit_enc_quarter(1, (i - 1) // 2)
            else:
                if i % 2 == 0 and b + 1 < B_L:
                    emit_enc_quarter(b + 1, i // 2)
                if encT_next is not None:
                    for g in range(2):
                        emit_transpose_group(encT_next, b + 1, i, g)
                if i == 3:
                    emit_ctx(b - 1)
            if tanh_prev is not None:
                ip, thp = tanh_prev
                emit_matvec(att_ps, ip, thp)
            th = tanhp.tile([P, 1024], BF16, tag="tanh")
            for c in range(2):
                nc.scalar.activation(
                    th[:, c * 512 : (c + 1) * 512],
                    ps[c][:],
                    AF.Tanh,
                    bias=U_hT[:, i, b : b + 1],
                    scale=1.0,
                )
            tanh_prev = (i, th)
        ip, thp = tanh_prev
        emit_matvec(att_ps, ip, thp)

        if b == 0:
            # enc[1] transposes: quarters arrive too late to interleave
            # into batch 0's slots; do them at the boundary.
            for k in range(NT):
                for g in range(2):
                    emit_transpose_group(encT_next, 1, k, g)
            masks.make_identity(nc, identf[:])
        emit_softmax_epilogue(b)
        if encT_next is not None:
            encT_cur = encT_next

    emit_ctx(B_L - 1)
    ctx.close()


_CACHED = None


def _build(stop_after="full"):
    global _CACHED
    if _CACHED is None:
        nc = bacc.Bacc("TRN2", target_bir_lowering=False, debug=False)
        with tile.TileContext(nc) as tc:
            _emit(tc, stop_after=stop_after)
        nc.compile()
        _CACHED = nc
    return _CACHED


def kernel(
    decoder_hidden: np.ndarray,
    encoder_outputs: np.ndarray,
    U_w: np.ndarray,
    W_w: np.ndarray,
    v_w: np.ndarray,
):
    dec = np.ascontiguousarray(np.asarray(decoder_hidden, dtype=np.float32))
    enc = np.ascontiguousarray(np.asarray(encoder_outputs, dtype=np.float32))
    U = np.ascontiguousarray(np.asarray(U_w, dtype=np.float32))
    W = np.ascontiguousarray(np.asarray(W_w, dtype=np.float32))
    v = np.ascontiguousarray(np.asarray(v_w, dtype=np.float32))

    nc = _build()
    in_maps = []
    for c in range(N_CORES):
        sl = slice(c * B_L, (c + 1) * B_L)
        in_maps.append(
            {"dec": dec[sl], "enc": enc[sl], "U_w": U, "W_w": W, "v_w": v}
        )
    res = run_bass_kernel_spmd(nc, in_maps, core_ids=list(range(N_CORES)))
    context = np.concatenate([res.results[c]["ctx"] for c in range(N_CORES)], axis=0)
    alpha = np.concatenate([res.results[c]["alpha"] for c in range(N_CORES)], axis=0)
    return (context.astype(np.float32), alpha.astype(np.float32))
